# revision 5
# baseline (speedup 1.0000x reference)
"""Physics-Attention Structured Mesh 3D — Trainium2 Bass kernel, 8 NeuronCores.

Sharding: spatial-parallel over the first grid axis (s0: 32 planes -> 8 slabs
of 4 planes each, halo 1 plane per side, zero-padded on the host). The
point-axis softmax is made shift-invariant with a static exponent shift M0,
so the only cross-core traffic is one AllReduce-add of the (E, S)
accumulators (66 KB) per batch element.

Step-1 structure (v2):
  - conv output xp kept resident in SBUF as bf16 (no DRAM round-trip)
  - input slabs loaded once per batch (not per co-tile)
  - E-matmul is group-local ([128n, 257] moving: 256 co + ones column, so
    S = sum_n e falls out of the same matmul)
  - recon+proj folded: final = e_row @ Q with Q = F @ out_w_head^T computed
    on-device from the AllReduced F (few tiny matmuls)
  - batch pipelining: conv(b+1) is emitted before post(b) so the AllReduce
    latency hides under the next batch's conv
"""
import numpy as np

import concourse.bacc as bacc
import concourse.mybir as mybir
import concourse.tile as tile
from concourse.bass_utils import run_bass_kernel_spmd
from concourse.masks import make_identity

F32 = mybir.dt.float32
F32R = mybir.dt.float32r
BF16 = mybir.dt.bfloat16
F16 = mybir.dt.float16
ALU = mybir.AluOpType
AX = mybir.AxisListType

NCORES = 8
B = 4
C = 256
INNER = 512
HH = 32
NLOC = 4 * HH * HH          # 4096 points per core
NCH = NLOC // 128           # 32 n-chunks
GROUPS = 2                  # co-tile pair-groups (4 heads each)
M0 = 60.0                   # static softmax exponent shift
LNLE = float(np.log(-np.log(np.float32(1e-6))))

_CACHE = {}


def _build():
    nc = bacc.Bacc("TRN2", target_bir_lowering=False, debug=False,
                   num_devices=NCORES)

    xt_d = nc.declare_dram_parameter("xt", [2, 128, B, 6, 34, 34], F32, isOutput=False)
    wst_d = nc.declare_dram_parameter("wst", [128, 54, 4, 128], F32, isOutput=False)
    cb_d = nc.declare_dram_parameter("cb", [128, 4], F32, isOutput=False)
    w2_d = nc.declare_dram_parameter("w2", [128, 66], F32, isOutput=False)
    brow_d = nc.declare_dram_parameter("brow", [1, 264], F32, isOutput=False)
    owq_d = nc.declare_dram_parameter("owq", [128, 4, 256], F32, isOutput=False)
    ob_d = nc.declare_dram_parameter("ob", [128, 2], F32, isOutput=False)
    out_d = nc.declare_dram_parameter("out", [B, 256, NLOC], F32, isOutput=True)

    es_in = [nc.dram_tensor(f"es_in{b}", [128, 130], F32) for b in range(B)]
    es_out = [nc.dram_tensor(f"es_out{b}", [128, 130], F32, addr_space="Shared")
              for b in range(B)]
    env = dict(xt_d=xt_d, wst_d=wst_d, cb_d=cb_d, w2_d=w2_d, brow_d=brow_d,
               owq_d=owq_d, ob_d=ob_d, out_d=out_d, es_in=es_in, es_out=es_out)

    with tile.TileContext(nc) as tc, \
         tc.tile_pool(name="konst", bufs=1) as konst, \
         tc.tile_pool(name="wstr", bufs=2) as wstrp, \
         tc.tile_pool(name="slab", bufs=12) as slabp, \
         tc.tile_pool(name="xp", bufs=4) as xpp, \
         tc.tile_pool(name="egt", bufs=4) as egtp, \
         tc.tile_pool(name="lts", bufs=4) as ltsp, \
         tc.tile_pool(name="est", bufs=4) as estp, \
         tc.tile_pool(name="xpt", bufs=4) as xptp, \
         tc.tile_pool(name="small", bufs=4) as small, \
         tc.tile_pool(name="qg", bufs=4) as qgp, \
         tc.tile_pool(name="outs", bufs=3) as outsp, \
         tc.tile_pool(name="psA", bufs=2, space="PSUM") as psA, \
         tc.tile_pool(name="psL", bufs=2, space="PSUM") as psL, \
         tc.tile_pool(name="psT", bufs=2, space="PSUM") as psT, \
         tc.tile_pool(name="psE", bufs=2, space="PSUM") as psE:

        _emit(nc, tc, env, konst, wstrp, slabp, xpp, egtp, ltsp, estp, xptp,
              small, qgp, outsp, psA, psL, psT, psE)

    nc.compile()
    return nc


def _conv_batch(nc, env, konst_t, wstrp, slabp, xpp, outsp, psA, b):
    """Direct 27-tap conv for batch b -> xp bf16 tiles [128co, 4096] x 4 cot.

    Returns list of 4 xp tiles (bf16, bias included)."""
    xt_d, wst_d = env["xt_d"], env["wst_d"]
    cbt = konst_t["cbt"]

    sl = {}
    for pl in range(6):
        for ch in range(2):
            st = slabp.tile([128, 34, 34], F32R, tag="sl",
                            name=f"sl_{b}_{ch}_{pl}")
            nc.sync.dma_start(st[:], xt_d.ap()[ch, :, b, pl, :, :].bitcast(F32R))
            sl[(ch, pl)] = st

    xps = []
    for cot in range(4):
        xpt_ = xpp.tile([128, NLOC], F16, tag="xp", name=f"xp_{b}_{cot}")
        xps.append(xpt_)
        wa = wstrp.tile([128, 27, 128], F32R, tag="wa", name=f"wa_{b}_{cot}")
        nc.sync.dma_start(wa[:], wst_d.ap()[:, 0:27, cot, :].bitcast(F32R))
        wb = wstrp.tile([128, 27, 128], F32R, tag="wb", name=f"wb_{b}_{cot}")
        nc.sync.dma_start(wb[:], wst_d.ap()[:, 27:54, cot, :].bitcast(F32R))
        for po in range(4):
            for hf in range(2):
                ps = psA.tile([128, 512], F32, tag="cps")
                for k in range(54):
                    t, ch = k // 2, k % 2
                    d0, d1, d2 = t // 9, (t // 3) % 3, t % 3
                    wt_ = wa if k < 27 else wb
                    rhs = sl[(ch, po + d0)][
                        :, 16 * hf + d1:16 * hf + d1 + 16, d2:d2 + 32]
                    nc.tensor.matmul(
                        ps[:], wt_[:, k % 27, :], rhs,
                        start=(k == 0), stop=(k == 53))
                n0 = po * 1024 + hf * 512
                nc.vector.tensor_scalar_add(xpt_[:, n0:n0 + 512], ps[:],
                                            cbt[:, cot:cot + 1])
    return xps


def _attn_batch(nc, env, konst_t, egtp, ltsp, estp, xptp, small, psL, psT,
                psE, xps, b):
    """Logits/temp/exp/transposes/E for batch b. Returns (egt list, )"""
    es_in, es_out = env["es_in"], env["es_out"]
    w2t, biasbc, identb, identh, m0b = (konst_t[k] for k in
        ("w2t", "biasbc", "identb", "identh", "m0b"))

    egt = [egtp.tile([128, NLOC], BF16, tag="egt", name=f"egt_{b}_{g_}")
           for g_ in range(GROUPS)]
    eps = [psE.tile([128, 257], F32, tag="eps", name=f"eps_{b}_{g_}")
           for g_ in range(GROUPS)]
    for jj in range(0, NCH, 2):
        ests = []
        for g in range(GROUPS):
            ps = psL.tile([128, 2, 132], F32, tag="lps")
            for dj in range(2):
                for a in range(2):
                    nc.tensor.matmul(
                        ps[:, dj, 66 * a:66 * a + 66],
                        xps[2 * g + a][:, 128 * (jj + dj):128 * (jj + dj) + 128],
                        w2t[:], start=True, stop=True)
            lts = ltsp.tile([128, 2, 4, 33], F32, tag="lts")
            nc.vector.tensor_tensor(
                lts[:], ps[:].rearrange("p a (b c) -> p a b c", b=4, c=33),
                biasbc[:], ALU.add)
            tt = small.tile([128, 2, 4], F32, tag="tt")
            nc.vector.tensor_scalar(tt[:], lts[:, :, :, 32],
                                    0.4, -0.4, ALU.min, ALU.max)
            nc.vector.tensor_scalar_add(tt[:], tt[:], 0.5)
            rt = small.tile([128, 2, 4], F32, tag="rt")
            nc.vector.reciprocal(rt[:], tt[:])
            est = estp.tile([128, 2, 4, 32], BF16, tag="est")
            nc.vector.tensor_tensor(
                lts[:, :, :, 0:32], lts[:, :, :, 0:32],
                rt[:].to_broadcast((128, 2, 4, 32)), ALU.mult)
            nc.scalar.activation(est[:], lts[:, :, :, 0:32],
                                 mybir.ActivationFunctionType.Exp,
                                 bias=m0b[:], scale=1.0)
            ests.append(est)
        for dj in range(2):
            j = jj + dj
            xpts = []
            for g in range(GROUPS):
                xpt = xptp.tile([128, 260], BF16, tag="xpt")
                for a in range(2):
                    pt = psT.tile([128, 128], F16, tag="tps")
                    nc.tensor.transpose(
                        pt[:], xps[2 * g + a][:, 128 * j:128 * j + 128],
                        identh[:])
                    nc.vector.tensor_copy(xpt[:, 128 * a:128 * a + 128], pt[:])
                nc.vector.memset(xpt[:, 256:257], 1.0)
                xpts.append(xpt)
            for g in range(GROUPS):
                echunk = ests[g][:, dj, :, :].rearrange("p b c -> p (b c)")
                pe_t = psT.tile([128, 128], BF16, tag="tps")
                nc.tensor.transpose(pe_t[:], echunk, identb[:])
                nc.vector.tensor_copy(egt[g][:, 128 * j:128 * j + 128],
                                      pe_t[:])
                nc.tensor.matmul(eps[g][:], echunk, xpts[g][:, 0:257],
                                 start=(j == 0), stop=(j == NCH - 1))

    # ---- pack E-diag + S ; AllReduce ----
    es2 = small.tile([128, 130], F32, tag="es2")
    for g in range(GROUPS):
        nc.vector.tensor_copy(es2[:, 65 * g + 64:65 * g + 65],
                              eps[g][:, 256:257])
        for k in range(4):
            nc.vector.tensor_copy(
                es2[32 * k:32 * k + 32, 65 * g:65 * g + 64],
                eps[g][32 * k:32 * k + 32, 64 * k:64 * k + 64])
    nc.sync.dma_start(es_in[b].ap(), es2[:])
    nc.gpsimd.collective_compute(
        "AllReduce", ALU.add,
        ins=[es_in[b].ap()], outs=[es_out[b].ap()],
        replica_groups=[list(range(NCORES))])
    return egt


def _post_batch(nc, env, konst_t, small, qgp, outsp, psL, psT, egt, b):
    """F -> Q -> final projection for batch b."""
    es_out, out_d = env["es_out"], env["out_d"]
    identr, owq, obt = (konst_t[k] for k in ("identr", "owq", "obt"))

    esr = small.tile([128, 130], F32, tag="esr")
    nc.sync.dma_start(esr[:], es_out[b].ap())

    # F = E * recip(S)^2 / (1+1e-5), packed as F2 [128, 2x64] then transposed
    f2 = small.tile([128, 128], F32R, tag="f2")
    for g in range(GROUPS):
        r1 = small.tile([128, 1], F32, tag="r1")
        nc.vector.reciprocal(r1[:], esr[:, 65 * g + 64:65 * g + 65])
        ft1 = small.tile([128, 64], F32, tag="ft1")
        nc.vector.tensor_scalar_mul(ft1[:], esr[:, 65 * g:65 * g + 64], r1[:])
        nc.vector.tensor_scalar(f2[:, 64 * g:64 * g + 64], ft1[:], r1[:],
                                1.0 / (1.0 + 1e-5), ALU.mult, ALU.mult)
    ftp = psT.tile([128, 128], F32R, tag="tps")
    nc.tensor.transpose(ftp[:], f2[:], identr[:])
    ft = small.tile([128, 128], F32R, tag="ftt")
    nc.vector.tensor_copy(ft[:], ftp[:])

    # Q[(hl,g32), d256] per group: 4 tiny matmuls each; PE can only write
    # PSUM stripes at base {0,32,64}, so drain each at base 0 and DMA-shift
    # (DMA crosses partitions) into the assembled Q tile
    qgs = []
    for g in range(GROUPS):
        qg = qgp.tile([128, 256], BF16, tag="qg", name=f"qg_{b}_{g}")
        for k in range(4):
            pq = psL.tile([128, 512], F32, tag="lps")
            nc.tensor.matmul(pq[0:32, 0:256],
                             ft[64 * g:64 * g + 64, 32 * k:32 * k + 32],
                             owq[64 * g:64 * g + 64, k, :],
                             start=True, stop=True)
            qt = small.tile([32, 256], BF16, tag="qt")
            nc.vector.tensor_copy(qt[:], pq[0:32, 0:256])
            nc.sync.dma_start(qg[32 * k:32 * k + 32, :], qt[:])
        qgs.append(qg)

    # final: out[128d, n] = sum_g Q_g[:,dchunk].T @ egt_g[:, nspan] + bias
    for w in range(8):
        for mt in range(2):
            po = psL.tile([128, 512], F32, tag="lps")
            for g in range(GROUPS):
                nc.tensor.matmul(po[:], qgs[g][:, 128 * mt:128 * mt + 128],
                                 egt[g][:, 512 * w:512 * w + 512],
                                 start=(g == 0), stop=(g == GROUPS - 1))
            osb = outsp.tile([128, 512], F32, tag="osb")
            nc.vector.tensor_scalar_add(osb[:], po[:], obt[:, mt:mt + 1])
            nc.sync.dma_start(
                out_d.ap()[b, 128 * mt:128 * mt + 128,
                           512 * w:512 * w + 512],
                osb[:])


def _emit(nc, tc, env, konst, wstrp, slabp, xpp, egtp, ltsp, estp, xptp,
          small, qgp, outsp, psA, psL, psT, psE):
    cb_d, w2_d, brow_d, owq_d, ob_d = (env[k] for k in
        ("cb_d", "w2_d", "brow_d", "owq_d", "ob_d"))

    # ---- constants ----
    cbt = konst.tile([128, 4], F32, tag="cbt")
    nc.sync.dma_start(cbt[:], cb_d.ap())
    w2f = konst.tile([128, 66], F32, tag="w2f")
    nc.sync.dma_start(w2f[:], w2_d.ap())
    w2t = konst.tile([128, 66], F16, tag="w2t")
    nc.vector.tensor_copy(w2t[:], w2f[:])
    biasbc = konst.tile([128, 2, 4, 33], F32, tag="biasbc")
    nc.sync.dma_start(biasbc[:].rearrange("p a b c -> p (a b c)"),
                      brow_d.ap().to_broadcast((128, 264)))
    ident = konst.tile([128, 128], F32, tag="ident")
    make_identity(nc, ident)
    identr = konst.tile([128, 128], F32R, tag="identr")
    nc.vector.tensor_copy(identr[:], ident[:])
    identb = konst.tile([128, 128], BF16, tag="identb")
    nc.vector.tensor_copy(identb[:], ident[:])
    identh = konst.tile([128, 128], F16, tag="identh")
    nc.vector.tensor_copy(identh[:], ident[:])
    owq = konst.tile([128, 4, 256], F32R, tag="owq")
    nc.sync.dma_start(owq[:], owq_d.ap().bitcast(F32R))
    obt = konst.tile([128, 2], F32, tag="obt")
    nc.sync.dma_start(obt[:], ob_d.ap())
    m0b = konst.tile([128, 1], F32, tag="m0b")
    nc.vector.memset(m0b[:], -M0)
    konst_t = dict(cbt=cbt, w2t=w2t, biasbc=biasbc, identr=identr,
                   identb=identb, identh=identh, owq=owq, obt=obt, m0b=m0b)

    # ---- pipelined batch loop ----
    egts = {}
    for b in range(B):
        xps = _conv_batch(nc, env, konst_t, wstrp, slabp, xpp, outsp, psA, b)
        if b - 1 in egts:
            _post_batch(nc, env, konst_t, small, qgp, outsp, psL, psT,
                        egts.pop(b - 1), b - 1)
        egts[b] = _attn_batch(nc, env, konst_t, egtp, ltsp, estp, xptp,
                              small, psL, psT, psE, xps, b)
    _post_batch(nc, env, konst_t, small, qgp, outsp, psL, psT,
                egts.pop(B - 1), B - 1)


def _prep_inputs(x, conv_w, conv_b, slice_w, slice_b, ada_w, ada_b, out_w, out_b):
    """Shard/transpose/pad the full inputs into 8 per-core input maps."""
    x = np.ascontiguousarray(x, np.float32)
    xT = np.zeros((C, B, 34, 34, 34), np.float32)
    xT[:, :, 1:33, 1:33, 1:33] = x.reshape(B, HH, HH, HH, C).transpose(4, 0, 1, 2, 3)

    # conv weights: [co, ci, 3,3,3] -> [ci%128, k=(t*2+ch), cot, co%128]
    wst = np.ascontiguousarray(
        conv_w.reshape(INNER, C, 27).transpose(1, 2, 0)      # [ci, t, co]
              .reshape(2, 128, 27, 4, 128)                   # [ch, ci, t, cot, co]
              .transpose(1, 2, 0, 3, 4)                      # [ci, t, ch, cot, co]
              .reshape(128, 54, 4, 128), np.float32)
    cb = np.ascontiguousarray(conv_b.reshape(4, 128).T, np.float32)

    w2 = np.zeros((128, 66), np.float32)
    w2[0:64, 0:32] = slice_w.T
    w2[0:64, 32] = ada_w[0]
    w2[64:128, 33:65] = slice_w.T
    w2[64:128, 65] = ada_w[0]

    bvec = np.concatenate([slice_b - LNLE, ada_b]).astype(np.float32)  # [33]
    brow = np.tile(bvec, 8).reshape(1, 264)

    # owq[64g+c, k, d] = out_w[d, 64*(4g+k)+c]
    owq = np.ascontiguousarray(
        out_w.T.reshape(2, 4, 64, 256).transpose(0, 2, 1, 3)
        .reshape(128, 4, 256), np.float32)
    ob = np.ascontiguousarray(out_b.reshape(2, 128).T, np.float32)

    in_maps = []
    for i in range(NCORES):
        slab = np.ascontiguousarray(xT[:, :, 4 * i:4 * i + 6, :, :]) \
            .reshape(2, 128, B, 6, 34, 34)
        in_maps.append({"xt": slab, "wst": wst, "cb": cb, "w2": w2,
                        "brow": brow, "owq": owq, "ob": ob})
    return in_maps


def kernel(**inputs):
    if "nc" not in _CACHE:
        _CACHE["nc"] = _build()
    nc = _CACHE["nc"]
    in_maps = _prep_inputs(
        np.asarray(inputs["x"]), np.asarray(inputs["conv_w"]),
        np.asarray(inputs["conv_b"]), np.asarray(inputs["slice_w"]),
        np.asarray(inputs["slice_b"]), np.asarray(inputs["ada_w"]),
        np.asarray(inputs["ada_b"]), np.asarray(inputs["out_w"]),
        np.asarray(inputs["out_b"]))
    res = run_bass_kernel_spmd(nc, in_maps, core_ids=list(range(NCORES)))
    out = np.empty((B, 32768, 256), np.float32)
    for i in range(NCORES):
        o = res.results[i]["out"]            # [B, 256, 4096]
        out[:, 4096 * i:4096 * (i + 1), :] = o.transpose(0, 2, 1)
    return out


# revision 13
# speedup vs baseline: 1.8262x; 1.8262x over previous
"""Physics-Attention Structured Mesh 3D — Trainium2 Bass kernel, 8 NeuronCores.

Sharding: spatial-parallel over the first grid axis (s0: 32 planes -> 8 slabs
of 4 planes each, halo 1 plane per side, zero-padded on the host). The
point-axis softmax is made shift-invariant with a static exponent shift M0,
so the only cross-core traffic is one AllReduce-add of the (E, S)
accumulators (66 KB) per batch element.

v3 structure:
  - conv = Winograd F(4,3) along the plane axis (each core's 4 output planes
    are exactly one F(4,3) tile): 27 taps -> 6 positions x 9 2D-taps, a 2x
    reduction in PE work. Input transform B^T runs on DVE+GpSimd (14 fused
    scalar_tensor_tensor ops per ci-half); the GEMM runs in fp16 (v and
    transformed weights u = G w); the A^T inverse transform runs on
    DVE (+2 ACT PSUM->SBUF copies) straight out of the 6 position PSUMs.
  - conv output xp kept resident in SBUF as fp16 (fp16, not bf16: the
    adaptive temperature divides logits by as little as 0.1, so logit noise
    is amplified ~10x in the softmax exponent)
  - e-path (exp values) in bf16 for fp32-class exponent range (values span
    e^-57..e^+; fp16 would underflow entire slices and break E/S)
  - E-matmul is group-local ([128n, 257] moving: 256 co + ones column, so
    S = sum_n e falls out of the same matmul)
  - recon+proj folded: final = e_row @ Q with Q = F @ out_w_head^T computed
    on-device from the AllReduced F (few tiny matmuls)
  - attention for head-group g (4 heads = co-tiles 2g,2g+1) is emitted right
    after its two co-tiles' conv, so attn overlaps the rest of the conv and
    the xp/PSUM pools rotate cleanly into the next batch; the per-batch
    AllReduce hides under the next batch's conv
"""
import numpy as np

import concourse.bacc as bacc
import concourse.mybir as mybir
import concourse.tile as tile
from concourse.bass_utils import run_bass_kernel_spmd
from concourse.masks import make_identity

F32 = mybir.dt.float32
F32R = mybir.dt.float32r
BF16 = mybir.dt.bfloat16
F16 = mybir.dt.float16
ALU = mybir.AluOpType
AX = mybir.AxisListType
ACTF = mybir.ActivationFunctionType

NCORES = 8
B = 4
C = 256
INNER = 512
HH = 32
NLOC = 4 * HH * HH          # 4096 points per core
NCH = NLOC // 128           # 32 n-chunks
GROUPS = 2                  # co-tile pair-groups (4 heads each)
M0 = 60.0                   # static softmax exponent shift
LNLE = float(np.log(-np.log(np.float32(1e-6))))

# F(4,3) weight transform (applied to conv_w on the host)
G43 = np.array([
    [1 / 4, 0, 0],
    [-1 / 6, -1 / 6, -1 / 6],
    [-1 / 6, 1 / 6, -1 / 6],
    [1 / 24, 1 / 12, 1 / 6],
    [1 / 24, -1 / 12, 1 / 6],
    [0, 0, 1]], np.float64)

_CACHE = {}


def _build():
    nc = bacc.Bacc("TRN2", target_bir_lowering=False, debug=False,
                   num_devices=NCORES)

    xt_d = nc.declare_dram_parameter("xt", [2, 128, B, 6, 34, 34], F32, isOutput=False)
    wq_d = nc.declare_dram_parameter("wq", [128, 6, 18, 4, 128], F16, isOutput=False)
    cb_d = nc.declare_dram_parameter("cb", [128, 4], F32, isOutput=False)
    w2_d = nc.declare_dram_parameter("w2", [128, 66], F32, isOutput=False)
    brow_d = nc.declare_dram_parameter("brow", [1, 264], F32, isOutput=False)
    owq_d = nc.declare_dram_parameter("owq", [128, 4, 256], F32, isOutput=False)
    ob_d = nc.declare_dram_parameter("ob", [128, 2], F32, isOutput=False)
    out_d = nc.declare_dram_parameter("out", [B, 256, NLOC], F32, isOutput=True)

    es_in = [nc.dram_tensor(f"es_in{b}", [128, 130], F32) for b in range(B)]
    es_out = [nc.dram_tensor(f"es_out{b}", [128, 130], F32, addr_space="Shared")
              for b in range(B)]
    env = dict(xt_d=xt_d, wq_d=wq_d, cb_d=cb_d, w2_d=w2_d, brow_d=brow_d,
               owq_d=owq_d, ob_d=ob_d, out_d=out_d, es_in=es_in, es_out=es_out)

    with tile.TileContext(nc) as tc, \
         tc.tile_pool(name="konst", bufs=1) as konst, \
         tc.tile_pool(name="wq", bufs=6) as wqp, \
         tc.tile_pool(name="slab", bufs=6) as slabp, \
         tc.tile_pool(name="vt", bufs=12) as vtp, \
         tc.tile_pool(name="tmp", bufs=3) as tmpp, \
         tc.tile_pool(name="minv", bufs=4) as minvp, \
         tc.tile_pool(name="rinv", bufs=5) as rinvp, \
         tc.tile_pool(name="xp", bufs=4) as xpp, \
         tc.tile_pool(name="egt", bufs=4) as egtp, \
         tc.tile_pool(name="lts", bufs=3) as ltsp, \
         tc.tile_pool(name="est", bufs=2) as estp, \
         tc.tile_pool(name="xpt", bufs=3) as xptp, \
         tc.tile_pool(name="small", bufs=4) as small, \
         tc.tile_pool(name="qg", bufs=4) as qgp, \
         tc.tile_pool(name="outs", bufs=2) as outsp, \
         tc.tile_pool(name="psA", bufs=3, space="PSUM") as psA, \
         tc.tile_pool(name="psL", bufs=2, space="PSUM") as psL, \
         tc.tile_pool(name="psT", bufs=1, space="PSUM") as psT, \
         tc.tile_pool(name="psE", bufs=2, space="PSUM") as psE:

        pools = dict(konst=konst, wqp=wqp, slabp=slabp, vtp=vtp, tmpp=tmpp,
                     minvp=minvp, rinvp=rinvp, xpp=xpp, egtp=egtp, ltsp=ltsp,
                     estp=estp, xptp=xptp, small=small, qgp=qgp, outsp=outsp,
                     psA=psA, psL=psL, psT=psT, psE=psE)
        _emit(nc, env, pools)

    nc.compile()
    return nc


def _transform_batch(nc, env, po, b):
    """Load x slabs for batch b and build the 12 F(4,3) B^T-transformed
    v tiles (fp16), alternating DVE/GpSimd."""
    xt_d = env["xt_d"]
    slabp, vtp, tmpp = po["slabp"], po["vtp"], po["tmpp"]

    vt = {}
    for ch in range(2):
        xs = []
        for pl in range(6):
            st = slabp.tile([128, 34, 34], F32, tag="sl",
                            name=f"sl_{b}_{ch}_{pl}")
            nc.sync.dma_start(st[:], xt_d.ap()[ch, :, b, pl, :, :])
            xs.append(st)
        vs = [vtp.tile([128, 34, 34], F16, tag="vt", name=f"vt_{b}_{ch}_{p}")
              for p in range(6)]
        for p in range(6):
            vt[(ch, p)] = vs[p]
        x0, x1, x2, x3, x4, x5 = (x[:] for x in xs)
        V, P = nc.vector, nc.gpsimd

        # GpSimd only supports plain 2-input ops; all fused stt stay on DVE
        # v0 = 4*x0 - 5*x2 + x4 ; v5 = 4*x1 - 5*x3 + x5
        t0 = tmpp.tile([128, 34, 34], F32, tag="tmp")
        V.scalar_tensor_tensor(t0[:], x0, 4.0, x4, ALU.mult, ALU.add)
        V.scalar_tensor_tensor(vs[0][:], x2, -5.0, t0[:], ALU.mult, ALU.add)
        t5 = tmpp.tile([128, 34, 34], F32, tag="tmp")
        V.scalar_tensor_tensor(t5[:], x1, 4.0, x5, ALU.mult, ALU.add)
        V.scalar_tensor_tensor(vs[5][:], x3, -5.0, t5[:], ALU.mult, ALU.add)
        # v1 = (x3+x4) - 4*(x1+x2) ; v2 = 4*(x1-x2) + (x4-x3)
        s1 = tmpp.tile([128, 34, 34], F32, tag="tmp")
        P.tensor_tensor(s1[:], x1, x2, ALU.add)
        s2 = tmpp.tile([128, 34, 34], F32, tag="tmp")
        P.tensor_tensor(s2[:], x3, x4, ALU.add)
        V.scalar_tensor_tensor(vs[1][:], s1[:], -4.0, s2[:], ALU.mult, ALU.add)
        d1 = tmpp.tile([128, 34, 34], F32, tag="tmp")
        P.tensor_tensor(d1[:], x1, x2, ALU.subtract)
        d2 = tmpp.tile([128, 34, 34], F32, tag="tmp")
        P.tensor_tensor(d2[:], x4, x3, ALU.subtract)
        V.scalar_tensor_tensor(vs[2][:], d1[:], 4.0, d2[:], ALU.mult, ALU.add)
        # v3 = 2*(x3-x1) + (x4-x2) ; v4 = (x4-x2) - 2*(x3-x1)
        d3 = tmpp.tile([128, 34, 34], F32, tag="tmp")
        P.tensor_tensor(d3[:], x3, x1, ALU.subtract)
        d4 = tmpp.tile([128, 34, 34], F32, tag="tmp")
        P.tensor_tensor(d4[:], x4, x2, ALU.subtract)
        V.scalar_tensor_tensor(vs[3][:], d3[:], 2.0, d4[:], ALU.mult, ALU.add)
        V.scalar_tensor_tensor(vs[4][:], d3[:], -2.0, d4[:], ALU.mult, ALU.add)
    return vt


def _conv_cot(nc, env, kt, po, vt, xps, b, cot):
    """Winograd GEMM + A^T inverse for one co-tile -> xps[cot] (fp16)."""
    wq_d = env["wq_d"]
    wqp, minvp, rinvp, xpp, psA = (po[k] for k in
                                   ("wqp", "minvp", "rinvp", "xpp", "psA"))
    cbt = kt["cbt"]

    xpt_ = xpp.tile([128, NLOC], F16, tag="xp", name=f"xp_{b}_{cot}")
    xps[cot] = xpt_
    wqt = []
    for pos in range(6):
        w_ = wqp.tile([128, 18, 128], F16, tag="wq",
                      name=f"wq_{b}_{cot}_{pos}")
        nc.sync.dma_start(w_[:], wq_d.ap()[:, pos, :, cot, :])
        wqt.append(w_)

    for hf in range(2):
        # inverse transform A^T (plane j <- 6 position accumulators):
        #   r0 = m0 + (m1+m2) + (m3+m4)
        #   r1 = (m1-m2) + 2(m3-m4)    r2 = (m1+m2) + 4(m3+m4)
        #   r3 = (m1-m2) + 8(m3-m4) + m5
        # Each position bank is drained immediately after its GEMM (ACT copy
        # or DVE combine) so at most ~2 psA banks are live — emitting all 6
        # before the inverse deadlocks the slot-limited scheduler.
        # DVE works with b2=m2-m1, dd=m4-m3 (negated forms).
        def oslice(j):
            n0 = j * 1024 + hf * 512
            return xpt_[:, n0:n0 + 512]

        tdr = {}
        comb = {}
        for pos in range(6):
            ps = psA.tile([128, 512], F32, tag="cps")
            for k in range(18):
                t, ch = k // 2, k % 2
                d1, d2 = t // 3, t % 3
                rhs = vt[(ch, pos)][
                    :, 16 * hf + d1:16 * hf + d1 + 16, d2:d2 + 32]
                nc.tensor.matmul(ps[:], wqt[pos][:, k, :], rhs,
                                 start=(k == 0), stop=(k == 17))
            if pos in (0, 1, 3):
                t_ = minvp.tile([128, 512], F32, tag="minv")
                nc.scalar.activation(t_[:], ps[:], ACTF.Copy)
                tdr[pos] = t_
            elif pos in (2, 4):
                tref = tdr[pos - 1]
                s_ = rinvp.tile([128, 512], F32, tag="rinv")
                nc.vector.tensor_tensor(s_[:], ps[:], tref[:], ALU.add)
                d_ = rinvp.tile([128, 512], F32, tag="rinv")
                nc.vector.tensor_tensor(d_[:], ps[:], tref[:], ALU.subtract)
                comb[pos] = (s_, d_)
                if pos == 4:
                    b2, dd = comb[2][1], d_
                    r1 = rinvp.tile([128, 512], F32, tag="rinv")
                    nc.vector.scalar_tensor_tensor(
                        r1[:], dd[:], -2.0, b2[:], ALU.mult, ALU.subtract)
                    nc.vector.tensor_scalar_add(oslice(1), r1[:],
                                                cbt[:, cot:cot + 1])
            else:  # pos 5: r3 = m5 + (-8*dd - b2), built in place
                b2, dd = comb[2][1], comb[4][1]
                r3 = rinvp.tile([128, 512], F32, tag="rinv")
                nc.vector.scalar_tensor_tensor(r3[:], dd[:], -8.0, b2[:],
                                               ALU.mult, ALU.subtract)
                nc.vector.tensor_tensor(r3[:], ps[:], r3[:], ALU.add)
                nc.vector.tensor_scalar_add(oslice(3), r3[:],
                                            cbt[:, cot:cot + 1])

        a_ = comb[2][0]
        c_ = comb[4][0]
        r0 = rinvp.tile([128, 512], F32, tag="rinv")
        nc.vector.tensor_tensor(r0[:], tdr[0][:], a_[:], ALU.add)
        nc.vector.tensor_tensor(r0[:], r0[:], c_[:], ALU.add)
        nc.vector.tensor_scalar_add(oslice(0), r0[:], cbt[:, cot:cot + 1])
        r2 = rinvp.tile([128, 512], F32, tag="rinv")
        nc.vector.scalar_tensor_tensor(r2[:], c_[:], 4.0, a_[:],
                                       ALU.mult, ALU.add)
        nc.vector.tensor_scalar_add(oslice(2), r2[:], cbt[:, cot:cot + 1])


def _attn_group(nc, env, kt, po, xps, b, g):
    """Logits/temp/exp/transposes/E for head-group g (co-tiles 2g, 2g+1)."""
    egtp, ltsp, estp, xptp, small, psL, psT, psE = (po[k] for k in
        ("egtp", "ltsp", "estp", "xptp", "small", "psL", "psT", "psE"))
    w2t, biasbc, identb, identh, m0b = (kt[k] for k in
        ("w2t", "biasbc", "identb", "identh", "m0b"))

    egt = egtp.tile([128, NLOC], BF16, tag="egt", name=f"egt_{b}_{g}")
    eps = psE.tile([128, 257], F32, tag="eps", name=f"eps_{b}_{g}")
    for jj in range(0, NCH, 2):
        ps = psL.tile([128, 2, 132], F32, tag="lps")
        for dj in range(2):
            for a in range(2):
                nc.tensor.matmul(
                    ps[:, dj, 66 * a:66 * a + 66],
                    xps[2 * g + a][:, 128 * (jj + dj):128 * (jj + dj) + 128],
                    w2t[:], start=True, stop=True)
        lts = ltsp.tile([128, 2, 4, 33], F32, tag="lts")
        nc.vector.tensor_tensor(
            lts[:], ps[:].rearrange("p a (b c) -> p a b c", b=4, c=33),
            biasbc[:], ALU.add)
        tt = small.tile([128, 2, 4], F32, tag="tt")
        nc.vector.tensor_scalar(tt[:], lts[:, :, :, 32],
                                0.4, -0.4, ALU.min, ALU.max)
        nc.vector.tensor_scalar_add(tt[:], tt[:], 0.5)
        rt = small.tile([128, 2, 4], F32, tag="rt")
        nc.vector.reciprocal(rt[:], tt[:])
        est = estp.tile([128, 2, 4, 32], BF16, tag="est")
        nc.vector.tensor_tensor(
            lts[:, :, :, 0:32], lts[:, :, :, 0:32],
            rt[:].to_broadcast((128, 2, 4, 32)), ALU.mult)
        nc.scalar.activation(est[:], lts[:, :, :, 0:32], ACTF.Exp,
                             bias=m0b[:], scale=1.0)
        for dj in range(2):
            j = jj + dj
            xpt = xptp.tile([128, 260], BF16, tag="xpt")
            for a in range(2):
                pt = psT.tile([128, 128], F16, tag="tps")
                nc.tensor.transpose(
                    pt[:], xps[2 * g + a][:, 128 * j:128 * j + 128],
                    identh[:])
                nc.vector.tensor_copy(xpt[:, 128 * a:128 * a + 128], pt[:])
            nc.vector.memset(xpt[:, 256:257], 1.0)
            echunk = est[:, dj, :, :].rearrange("p b c -> p (b c)")
            pe_t = psT.tile([128, 128], BF16, tag="tps")
            nc.tensor.transpose(pe_t[:], echunk, identb[:])
            nc.vector.tensor_copy(egt[:, 128 * j:128 * j + 128], pe_t[:])
            nc.tensor.matmul(eps[:], echunk, xpt[:, 0:257],
                             start=(j == 0), stop=(j == NCH - 1))
    return egt, eps


def _pack_collective(nc, env, po, epss, b):
    small = po["small"]
    es_in, es_out = env["es_in"], env["es_out"]
    es2 = small.tile([128, 130], F32, tag="es2")
    for g in range(GROUPS):
        nc.vector.tensor_copy(es2[:, 65 * g + 64:65 * g + 65],
                              epss[g][:, 256:257])
        for k in range(4):
            nc.vector.tensor_copy(
                es2[32 * k:32 * k + 32, 65 * g:65 * g + 64],
                epss[g][32 * k:32 * k + 32, 64 * k:64 * k + 64])
    nc.sync.dma_start(es_in[b].ap(), es2[:])
    nc.gpsimd.collective_compute(
        "AllReduce", ALU.add,
        ins=[es_in[b].ap()], outs=[es_out[b].ap()],
        replica_groups=[list(range(NCORES))])


def _post_batch(nc, env, kt, po, egt, b):
    """F -> Q -> final projection for batch b."""
    es_out, out_d = env["es_out"], env["out_d"]
    small, qgp, outsp, psL, psT = (po[k] for k in
                                   ("small", "qgp", "outsp", "psL", "psT"))
    identr, owq, obt = (kt[k] for k in ("identr", "owq", "obt"))

    esr = small.tile([128, 130], F32, tag="esr")
    nc.sync.dma_start(esr[:], es_out[b].ap())

    # F = E * recip(S)^2 / (1+1e-5), packed as F2 [128, 2x64] then transposed
    f2 = small.tile([128, 128], F32R, tag="f2")
    for g in range(GROUPS):
        r1 = small.tile([128, 1], F32, tag="r1")
        nc.vector.reciprocal(r1[:], esr[:, 65 * g + 64:65 * g + 65])
        ft1 = small.tile([128, 64], F32, tag="ft1")
        nc.vector.tensor_scalar_mul(ft1[:], esr[:, 65 * g:65 * g + 64], r1[:])
        nc.vector.tensor_scalar(f2[:, 64 * g:64 * g + 64], ft1[:], r1[:],
                                1.0 / (1.0 + 1e-5), ALU.mult, ALU.mult)
    ftp = psT.tile([128, 128], F32R, tag="tps")
    nc.tensor.transpose(ftp[:], f2[:], identr[:])
    ft = small.tile([128, 128], F32R, tag="ftt")
    nc.vector.tensor_copy(ft[:], ftp[:])

    # Q[(hl,g32), d256] per group: 4 tiny matmuls each; PE can only write
    # PSUM stripes at base {0,32,64}, so drain each at base 0 and DMA-shift
    # (DMA crosses partitions) into the assembled Q tile
    qgs = []
    for g in range(GROUPS):
        qg = qgp.tile([128, 256], BF16, tag="qg", name=f"qg_{b}_{g}")
        for k in range(4):
            pq = psL.tile([128, 512], F32, tag="lps")
            nc.tensor.matmul(pq[0:32, 0:256],
                             ft[64 * g:64 * g + 64, 32 * k:32 * k + 32],
                             owq[64 * g:64 * g + 64, k, :],
                             start=True, stop=True)
            qt = small.tile([32, 256], BF16, tag="qt")
            nc.vector.tensor_copy(qt[:], pq[0:32, 0:256])
            nc.sync.dma_start(qg[32 * k:32 * k + 32, :], qt[:])
        qgs.append(qg)

    # final: out[128d, n] = sum_g Q_g[:,dchunk].T @ egt_g[:, nspan] + bias
    for w in range(8):
        for mt in range(2):
            po_ = psL.tile([128, 512], F32, tag="lps")
            for g in range(GROUPS):
                nc.tensor.matmul(po_[:], qgs[g][:, 128 * mt:128 * mt + 128],
                                 egt[g][:, 512 * w:512 * w + 512],
                                 start=(g == 0), stop=(g == GROUPS - 1))
            osb = outsp.tile([128, 512], F32, tag="osb")
            nc.vector.tensor_scalar_add(osb[:], po_[:], obt[:, mt:mt + 1])
            nc.sync.dma_start(
                out_d.ap()[b, 128 * mt:128 * mt + 128,
                           512 * w:512 * w + 512],
                osb[:])


def _emit(nc, env, po):
    konst = po["konst"]
    cb_d, w2_d, brow_d, owq_d, ob_d = (env[k] for k in
        ("cb_d", "w2_d", "brow_d", "owq_d", "ob_d"))

    # ---- constants ----
    cbt = konst.tile([128, 4], F32, tag="cbt")
    nc.sync.dma_start(cbt[:], cb_d.ap())
    w2f = konst.tile([128, 66], F32, tag="w2f")
    nc.sync.dma_start(w2f[:], w2_d.ap())
    w2t = konst.tile([128, 66], F16, tag="w2t")
    nc.vector.tensor_copy(w2t[:], w2f[:])
    biasbc = konst.tile([128, 2, 4, 33], F32, tag="biasbc")
    nc.sync.dma_start(biasbc[:].rearrange("p a b c -> p (a b c)"),
                      brow_d.ap().to_broadcast((128, 264)))
    ident = konst.tile([128, 128], F32, tag="ident")
    make_identity(nc, ident)
    identr = konst.tile([128, 128], F32R, tag="identr")
    nc.vector.tensor_copy(identr[:], ident[:])
    identb = konst.tile([128, 128], BF16, tag="identb")
    nc.vector.tensor_copy(identb[:], ident[:])
    identh = konst.tile([128, 128], F16, tag="identh")
    nc.vector.tensor_copy(identh[:], ident[:])
    owq = konst.tile([128, 4, 256], F32R, tag="owq")
    nc.sync.dma_start(owq[:], owq_d.ap().bitcast(F32R))
    obt = konst.tile([128, 2], F32, tag="obt")
    nc.sync.dma_start(obt[:], ob_d.ap())
    m0b = konst.tile([128, 1], F32, tag="m0b")
    nc.vector.memset(m0b[:], -M0)
    kt = dict(cbt=cbt, w2t=w2t, biasbc=biasbc, identr=identr,
              identb=identb, identh=identh, owq=owq, obt=obt, m0b=m0b)

    # ---- pipelined batch loop ----
    pending = {}       # b -> (egt list, eps list)
    for b in range(B):
        vt = _transform_batch(nc, env, po, b)
        xps = [None] * 4
        egts, epss = [], []
        for g in range(GROUPS):
            _conv_cot(nc, env, kt, po, vt, xps, b, 2 * g)
            _conv_cot(nc, env, kt, po, vt, xps, b, 2 * g + 1)
            eg, ep = _attn_group(nc, env, kt, po, xps, b, g)
            egts.append(eg)
            epss.append(ep)
            if g == 0 and (b - 1) in pending:
                _post_batch(nc, env, kt, po, pending.pop(b - 1), b - 1)
        _pack_collective(nc, env, po, epss, b)
        pending[b] = egts
    _post_batch(nc, env, kt, po, pending.pop(B - 1), B - 1)


def _prep_inputs(x, conv_w, conv_b, slice_w, slice_b, ada_w, ada_b, out_w, out_b):
    """Shard/transpose/pad the full inputs into 8 per-core input maps."""
    x = np.ascontiguousarray(x, np.float32)
    xT = np.zeros((C, B, 34, 34, 34), np.float32)
    xT[:, :, 1:33, 1:33, 1:33] = x.reshape(B, HH, HH, HH, C).transpose(4, 0, 1, 2, 3)

    # Winograd F(4,3) weights: u[pos,y,x,ci,co] = sum_dz G[pos,dz] w[co,ci,dz,y,x]
    u = np.einsum('pz,oizyx->pyxio', G43, conv_w.astype(np.float64))
    wq = np.ascontiguousarray(
        u.reshape(6, 9, 2, 128, 4, 128).transpose(3, 0, 1, 2, 4, 5)
         .reshape(128, 6, 18, 4, 128), np.float16)
    cb = np.ascontiguousarray(conv_b.reshape(4, 128).T, np.float32)

    w2 = np.zeros((128, 66), np.float32)
    w2[0:64, 0:32] = slice_w.T
    w2[0:64, 32] = ada_w[0]
    w2[64:128, 33:65] = slice_w.T
    w2[64:128, 65] = ada_w[0]

    bvec = np.concatenate([slice_b - LNLE, ada_b]).astype(np.float32)  # [33]
    brow = np.tile(bvec, 8).reshape(1, 264)

    # owq[64g+c, k, d] = out_w[d, 64*(4g+k)+c]
    owq = np.ascontiguousarray(
        out_w.T.reshape(2, 4, 64, 256).transpose(0, 2, 1, 3)
        .reshape(128, 4, 256), np.float32)
    ob = np.ascontiguousarray(out_b.reshape(2, 128).T, np.float32)

    in_maps = []
    for i in range(NCORES):
        slab = np.ascontiguousarray(xT[:, :, 4 * i:4 * i + 6, :, :]) \
            .reshape(2, 128, B, 6, 34, 34)
        in_maps.append({"xt": slab, "wq": wq, "cb": cb, "w2": w2,
                        "brow": brow, "owq": owq, "ob": ob})
    return in_maps


def kernel(**inputs):
    if "nc" not in _CACHE:
        _CACHE["nc"] = _build()
    nc = _CACHE["nc"]
    in_maps = _prep_inputs(
        np.asarray(inputs["x"]), np.asarray(inputs["conv_w"]),
        np.asarray(inputs["conv_b"]), np.asarray(inputs["slice_w"]),
        np.asarray(inputs["slice_b"]), np.asarray(inputs["ada_w"]),
        np.asarray(inputs["ada_b"]), np.asarray(inputs["out_w"]),
        np.asarray(inputs["out_b"]))
    res = run_bass_kernel_spmd(nc, in_maps, core_ids=list(range(NCORES)))
    out = np.empty((B, 32768, 256), np.float32)
    for i in range(NCORES):
        o = res.results[i]["out"]            # [B, 256, 4096]
        out[:, 4096 * i:4096 * (i + 1), :] = o.transpose(0, 2, 1)
    return out
